# revision 17
# baseline (speedup 1.0000x reference)
"""Multi-head linear attention on Trainium2 — 8-core SPMD, batch+head sharded.

Full-tensor contract: kernel(**inputs) takes the complete Q/K/V
[4, 4096, 1024] f32 arrays, internally shards them across 8 NeuronCores
(core c -> batch c//2, heads 8*(c%2) .. 8*(c%2)+8, i.e. a contiguous
512-column slice of the embedding dim), runs one Bass kernel per core,
and reassembles the full [4, 4096, 1024] f32 output.

Per-core math (H=8 local heads, D=64, L=4096):
    phi = sigmoid(0.6053*x - 4.102)
    kv_ext[h] = phi_K[h]^T @ [V[h] | 1]     # [64, 65], f32 PSUM accum
    numden[h] = phi_Q[h] @ kv_ext[h]        # [L, 65]
    out[h]    = numden[h][:, :64] / numden[h][:, 64:65]

v2 design, balanced around four ~27-29us per-core resource floors:
  * DMA: Q/K/V inputs are fp8 e3m4 (host casts; rel err ~1.5e-2 vs the
    2e-2 gate), output fp16 -> 10.5 MB/core @ ~360 GB/s = 29us.
  * Scalar (Activation, 1.2 GHz): 4.2M sigmoids = 27.4us. It does ZERO
    DMA dispatch; loads go on the sync/vector HWDGE queues + gpsimd
    SWDGE, stores on sync/gpsimd.
  * PE: 256x (LDWEIGHTS 128 + 130 cols) = 27.5us.
  * DVE: per 2048-q piece, division = 2 batched reciprocals + 2
    multiplies using multi-bank PSUM num tiles (12 blocks in 4 banks +
    4 blocks in 2 banks, 5-dim APs) ~ 24us.

The 8 heads form 4 PAIRS, processed as a pipeline: pair g's
kv-accumulation streams while pair g-1's Q phase computes.  K and V are
merged row-wise on the host into [K_pair(128)|V_pair(128)|1|1] fp8 rows
pre-permuted into exact SBUF tile order so every DMA is a sequential
sweep with 4128 B descriptors (pairs 1-3: 2 batches of 8 tiles; pair 0:
4 batches of 4 tiles for a faster pipeline ramp).  One [128,130] matmul
per 128-row chunk (fp16 phi x fp8 V|1|1) accumulates kv AND k_sum.
phi_Q is computed per 2048-col piece so the tail pair's Q matmuls chase
the activation.  kv for a pair packs into a block-diagonal [128, 130]
fp16 operand; a 128-q matmul against it yields both heads' num|den.
"""

import numpy as np

B = 4
L = 4096
E = 1024
NH = 8            # heads per core
D = 64
W = D + 1         # head block width incl. ones/den column
EC = NH * D       # 512 embedding columns per core
P = 128
G = 4             # head pairs, stacked along rows (pipeline depth)
GC = EC // G      # 128 columns per pair
SUB = 2           # L-rows per partition line
VW = 2 * W        # 130: V|1|1 block width
KVW = GC + VW     # 258: merged K|V|1|1 staged row width
NT = L // (P * SUB)   # 16 tiles (256 L-rows) per pair
TB0 = 4           # tiles per batch, pair 0 (fast ramp)
NB0 = NT // TB0   # 4 batches for pair 0
TBS = 8           # tiles per batch, pairs 1-3 (4128 B descriptors)
NBS = NT // TBS   # 2 batches per pair
QB = 2048         # q columns per Q-phase piece
NQB = L // QB     # 2 pieces per pair
N_CORES = 8

_CACHE = {}


def _build_nc():
    from contextlib import ExitStack

    import concourse.bacc as bacc
    import concourse.bass as bass
    import concourse.mybir as mybir
    import concourse.tile as tile

    f32 = mybir.dt.float32
    f16 = mybir.dt.float16
    f8 = mybir.dt.float8e3
    SIG = mybir.ActivationFunctionType.Sigmoid

    nc = bacc.Bacc("TRN2", target_bir_lowering=False, debug=False)
    # DRAM tensors are staged by the host in exact SBUF-tile order so every
    # DMA is a fully sequential sweep with maximal descriptors.
    Q = nc.dram_tensor("Q", [EC, L], f8, kind="ExternalInput").ap()
    KV0 = nc.dram_tensor("KV0", [NB0 * P, TB0 * SUB * KVW], f8,
                         kind="ExternalInput").ap()
    KVR = nc.dram_tensor("KV", [(G - 1) * NBS * P, TBS * SUB * KVW], f8,
                         kind="ExternalInput").ap()
    O = nc.dram_tensor("O", [G * NQB * P, QB], f16, kind="ExternalOutput").ap()

    def sub_ap(t, off, dims):
        return bass.AP(tensor=t.tensor, offset=t.offset + off,
                       ap=[t.ap[0]] + dims)

    with tile.TileContext(nc) as tc, ExitStack() as ctx:
        singles = ctx.enter_context(tc.tile_pool(name="singles", bufs=1))
        ldr = ctx.enter_context(tc.tile_pool(name="ldr", bufs=6))
        ph0 = ctx.enter_context(tc.tile_pool(name="ph0", bufs=1))
        phr = ctx.enter_context(tc.tile_pool(name="phr", bufs=2))
        rcp = ctx.enter_context(tc.tile_pool(name="rcp", bufs=2))
        ob = ctx.enter_context(tc.tile_pool(name="ob", bufs=3))
        # PSUM: 2 banks kv accum + 4-bank and 2-bank num tiles = 8 banks.
        pkv = ctx.enter_context(tc.tile_pool(name="pkv", bufs=2, space="PSUM"))
        pna = ctx.enter_context(tc.tile_pool(name="pna", bufs=1, space="PSUM"))
        pnb = ctx.enter_context(tc.tile_pool(name="pnb", bufs=1, space="PSUM"))

        sig_bias = singles.tile([P, 1], f32)
        nc.vector.memset(sig_bias, -4.102)

        # Block-diagonal kv operand per head pair: rows 0:64 cols 0:65 hold
        # kv_ext of the even head, rows 64:128 cols 65:130 the odd head.
        kv_bd = singles.tile([P, G, VW], f16)
        nc.vector.memset(kv_bd, 0.0)

        # Single Q staging tile so the load is 2 coarse scalar-queue
        # dispatches (pairs 0-1, pairs 2-3) costing ~1.8us of scalar
        # sequencer time instead of 4x ~0.9us.
        q_raw = singles.tile([P, G, L], f8)
        phiq = [singles.tile([P, L], f16, tag=f"pq{g}", name=f"pq{g}")
                for g in range(G)]
        # Pair 0's four KV batches share one tile: one 4096-elem phiK
        # activation covers the whole pair (PE is idle until it anyway).
        kvt0 = singles.tile([P, NB0, TB0, SUB, KVW], f8)

        kv_tiles = {}
        kv_ps = {}

        def emit_q_load(half):
            # Two coarse dispatches (pairs 0-1, 2-3) keep the scalar
            # sequencer cost low; they queue BEHIND kvr(1,0) on the
            # scalar HWDGE queue so the ramp-critical KV batch wins.
            nc.scalar.dma_start(
                out=q_raw[:, 2 * half:2 * half + 2, :],
                in_=Q[2 * half * P:(2 * half + 2) * P, :].rearrange(
                    "(k p) l -> p k l", k=2))

        def emit_kv_dispatch0(b):
            # All of pair 0 on sync: the gpsimd SWDGE queue starts ~2us
            # later than HWDGE and pair 0 is ramp-critical.
            nc.sync.dma_start(
                out=kvt0[:, b, :, :, :],
                in_=KV0[b * P:(b + 1) * P, :].rearrange(
                    "p (t s e) -> p t s e", t=TB0, s=SUB))

        def emit_kv_dispatch_r(g, ib):
            r0 = ((g - 1) * NBS + ib) * P
            kvt = ldr.tile([P, TBS, SUB, KVW], f8, tag="kvtr", name="kvtr")
            # (1,0) rides the otherwise-idle scalar queue ahead of the Q
            # stream so pair-1 compute starts right after pair 0.
            if (g, ib) == (1, 0):
                eng = nc.scalar
            else:
                eng = nc.sync if ib == 0 else nc.gpsimd
            eng.dma_start(
                out=kvt,
                in_=KVR[r0:r0 + P, :].rearrange(
                    "p (t s e) -> p t s e", t=TBS, s=SUB))
            kv_tiles[(g, ib)] = kvt

        def emit_kv_compute0(half):
            # One activation per two batches (they land back-to-back on
            # the same queue; fewer instructions on the scalar chain).
            b0 = 2 * half
            phiK = ph0.tile([P, 2, TB0, SUB, GC], f16, tag="phiK0",
                            name="phiK0", bufs=2)
            nc.scalar.activation(
                out=phiK, in_=kvt0[:, b0:b0 + 2, :, :, 0:GC], func=SIG,
                bias=sig_bias, scale=0.6053)
            for i in range(2):
                for t in range(TB0):
                    for s in range(SUB):
                        nc.tensor.matmul(
                            out=kv_ps[0][:, 0:VW],
                            lhsT=phiK[:, i, t, s, :],
                            rhs=kvt0[:, b0 + i, t, s, GC:KVW],
                            start=(b0 + i == 0 and t == 0 and s == 0),
                            stop=(b0 + i == NB0 - 1 and t == TB0 - 1
                                  and s == SUB - 1))

        def emit_kv_compute(g, ib):
            kvt = kv_tiles.pop((g, ib))
            phiK = phr.tile([P, TBS, SUB, GC], f16, tag="phiKr", name="phiK")
            nc.scalar.activation(
                out=phiK, in_=kvt[:, :, :, 0:GC], func=SIG, bias=sig_bias,
                scale=0.6053)
            for t in range(TBS):
                for s in range(SUB):
                    nc.tensor.matmul(
                        out=kv_ps[g][:, 0:VW],
                        lhsT=phiK[:, t, s, :],
                        rhs=kvt[:, t, s, GC:KVW],
                        start=(ib == 0 and t == 0 and s == 0),
                        stop=(ib == NBS - 1 and t == TBS - 1
                              and s == SUB - 1))

        def emit_phiq(g, qb=None):
            # qb=None: whole pair in one activation (pairs 0-2); pair 3
            # stays split per piece so its Q matmuls chase the activation.
            sl = slice(0, L) if qb is None else slice(qb * QB, (qb + 1) * QB)
            nc.scalar.activation(
                out=phiq[g][:, sl], in_=q_raw[:, g, sl],
                func=SIG, bias=sig_bias, scale=0.6053)

        def emit_kv_finish(g):
            """Pack the pair's kv PSUM tile into the block-diag operand."""
            ps = kv_ps.pop(g)
            nc.vector.tensor_copy(out=kv_bd[0:D, g, 0:D], in_=ps[0:D, 0:D])
            nc.vector.tensor_copy(
                out=kv_bd[0:D, g, D:W], in_=ps[0:D, 2 * D:2 * D + 1])
            nc.vector.tensor_copy(
                out=kv_bd[D:P, g, W:W + D], in_=ps[D:P, D:2 * D])
            nc.vector.tensor_copy(
                out=kv_bd[D:P, g, W + D:VW], in_=ps[D:P, 2 * D:2 * D + 1])

        def emit_q_piece(g, qb, split_store=False):
            """16 q-block matmuls + batched recip/mult + store.

            num blocks live 3-per-bank in a 4-bank (12 blocks) and a
            2-bank (4 blocks, 2 per bank) PSUM tile; one reciprocal and
            one 5-dim-AP multiply covers each tile."""
            pi = g * NQB + qb
            q0 = qb * QB
            orow = pi * P
            out_t = ob.tile([P, QB], f16, tag="outt", name="out_t")

            numA = pna.tile([P, 2048], f32, tag="numA", name="numA")
            for blk in range(12):
                k, j = divmod(blk, 3)
                nc.tensor.matmul(
                    out=sub_ap(numA, 512 * k + 130 * j, [[1, VW]]),
                    lhsT=phiq[g][:, q0 + blk * P:q0 + (blk + 1) * P],
                    rhs=kv_bd[:, g, :], start=True, stop=True)
            rA = rcp.tile([P, 4, 3, 2], f32, tag="rA", name="rA")
            nc.vector.reciprocal(
                out=rA, in_=sub_ap(numA, D, [[512, 4], [130, 3], [W, 2]]))
            nc.vector.tensor_tensor(
                out=sub_ap(out_t, 0, [[384, 4], [128, 3], [64, 2], [1, 64]]),
                in0=sub_ap(numA, 0, [[512, 4], [130, 3], [W, 2], [1, D]]),
                in1=sub_ap(rA, 0, [[6, 4], [2, 3], [1, 2], [0, D]]),
                op=mybir.AluOpType.mult)
            if split_store:
                engA = nc.sync if pi % 2 == 0 else nc.gpsimd
                engA.dma_start(out=O[orow:orow + P, 0:1536],
                               in_=out_t[:, 0:1536])

            numB = pnb.tile([P, 1024], f32, tag="numB", name="numB")
            for blk in range(12, 16):
                k, j = divmod(blk - 12, 2)
                nc.tensor.matmul(
                    out=sub_ap(numB, 512 * k + 130 * j, [[1, VW]]),
                    lhsT=phiq[g][:, q0 + blk * P:q0 + (blk + 1) * P],
                    rhs=kv_bd[:, g, :], start=True, stop=True)
            rB = rcp.tile([P, 2, 2, 2], f32, tag="rB", name="rB")
            nc.vector.reciprocal(
                out=rB, in_=sub_ap(numB, D, [[512, 2], [130, 2], [W, 2]]))
            nc.vector.tensor_tensor(
                out=sub_ap(out_t, 1536, [[256, 2], [128, 2], [64, 2], [1, 64]]),
                in0=sub_ap(numB, 0, [[512, 2], [130, 2], [W, 2], [1, D]]),
                in1=sub_ap(rB, 0, [[4, 2], [2, 2], [1, 2], [0, D]]),
                op=mybir.AluOpType.mult)
            if split_store:
                engB = nc.gpsimd if pi % 2 == 0 else nc.sync
                engB.dma_start(out=O[orow:orow + P, 1536:QB],
                               in_=out_t[:, 1536:QB])
            else:
                eng = nc.sync if pi % 2 == 0 else nc.gpsimd
                eng.dma_start(out=O[orow:orow + P, :], in_=out_t)

        # Dummy sigmoid on the bias column: pulls the framework's
        # activation-table load to the head of the scalar stream where it
        # overlaps the idle ramp (the load is async; only the first
        # dependent activation waits on it).
        warm = singles.tile([P, 1], f32)
        nc.scalar.activation(out=warm, in_=sig_bias, func=SIG,
                             bias=sig_bias, scale=1.0)

        # ---- load dispatches.  Per-engine streams:
        #   scalar: kvr(1,0), QA (pairs 0-1), QB (pairs 2-3)
        #   sync:   kv0 b0/b2, kvr(2,0), kvr(3,0), even stores
        #   gpsimd: kv0 b1/b3, kvr(1,1), kvr(2,1), kvr(3,1), odd stores
        emit_kv_dispatch_r(1, 0)
        emit_q_load(0)
        emit_q_load(1)
        for b in range(NB0):
            emit_kv_dispatch0(b)
        emit_kv_dispatch_r(1, 1)
        for g in range(2, G):
            for ib in range(NBS):
                emit_kv_dispatch_r(g, ib)

        # ---- software-pipelined compute: pair g's kv streams while
        # pair g-1's Q phase computes.  phiQ(g-1) interleaves with pair
        # g's phiK so the scalar stream never starves the PE. ----
        kv_ps[0] = pkv.tile([P, 512], f32, tag="kv", name="kv")
        emit_kv_compute0(0)
        emit_kv_compute0(1)
        emit_kv_finish(0)
        for g in range(1, G):
            kv_ps[g] = pkv.tile([P, 512], f32, tag="kv", name="kv")
            emit_kv_compute(g, 0)
            emit_phiq(g - 1)
            emit_q_piece(g - 1, 0)
            emit_kv_compute(g, 1)
            emit_q_piece(g - 1, 1)
            emit_kv_finish(g)
        emit_phiq(G - 1, 0)
        emit_q_piece(G - 1, 0, split_store=True)
        emit_phiq(G - 1, 1)
        emit_q_piece(G - 1, 1, split_store=True)

    nc.compile()
    return nc


def _get_nc():
    if "nc" not in _CACHE:
        _CACHE["nc"] = _build_nc()
    return _CACHE["nc"]


def _f8():
    import ml_dtypes
    return ml_dtypes.float8_e3m4


def _shard_q(arr):
    """Full [B, L, E] f32 -> per-core transposed [512, L] fp8 slices."""
    f8 = _f8()
    out = []
    for c in range(N_CORES):
        b, g = divmod(c, 2)
        out.append(np.ascontiguousarray(
            arr[b, :, g * EC:(g + 1) * EC].T.astype(f8)))
    return out


def _shard_kv(karr, varr):
    """Full K/V [B, L, E] f32 -> per-core (KV0, KVR) fp8 staging: merged
    [K_pair(128) | V_pair(128) | 1 | 1] rows pre-permuted into SBUF tile
    order [ib][p][t][s][e] so every batch DMA is one sequential sweep."""
    f8 = _f8()
    out = []
    for c in range(N_CORES):
        b, g = divmod(c, 2)
        ksl = karr[b, :, g * EC:(g + 1) * EC]
        vsl = varr[b, :, g * EC:(g + 1) * EC]
        st = np.ones((G, L, KVW), dtype=f8)
        for pg in range(G):
            st[pg, :, 0:GC] = ksl[:, pg * GC:(pg + 1) * GC].astype(f8)
            st[pg, :, GC:GC + P] = vsl[:, pg * P:(pg + 1) * P].astype(f8)
        # row (ib, t, p, s) -> staged position (ib, p, t, s)
        kv0 = st[0].reshape(NB0, TB0, P, SUB, KVW).transpose(
            0, 2, 1, 3, 4).reshape(NB0 * P, TB0 * SUB * KVW)
        kvr = st[1:].reshape(G - 1, NBS, TBS, P, SUB, KVW).transpose(
            0, 1, 3, 2, 4, 5).reshape((G - 1) * NBS * P, TBS * SUB * KVW)
        out.append((np.ascontiguousarray(kv0), np.ascontiguousarray(kvr)))
    return out


def _unshard_o(o):
    """Per-core [4*2*128, 2048] fp16 (piece-major) -> [L, EC] f32 slice."""
    blocks = np.asarray(o).reshape(G, NQB, P, QB // P, P)  # [pg, qb, p, qk, e]
    # q = qb*QB + qk*P + p
    perm = blocks.transpose(0, 1, 3, 2, 4).reshape(G, L, P)
    return np.concatenate(list(perm), axis=1).astype(np.float32)


def make_in_maps(inputs):
    Q = np.asarray(inputs["Q"], dtype=np.float32)
    K = np.asarray(inputs["K"], dtype=np.float32)
    V = np.asarray(inputs["V"], dtype=np.float32)
    qs, kvs = _shard_q(Q), _shard_kv(K, V)
    return [{"Q": qs[c], "KV0": kvs[c][0], "KV": kvs[c][1]}
            for c in range(N_CORES)]


def run_sharded(in_maps, trace=False, trace_cores=None):
    from concourse.bass_utils import run_bass_kernel_spmd

    nc = _get_nc()
    kwargs = {}
    if trace:
        kwargs = dict(trace=True, trace_cores=trace_cores or [0])
    return run_bass_kernel_spmd(nc, in_maps, core_ids=list(range(N_CORES)),
                                **kwargs)


def kernel(**inputs):
    in_maps = make_in_maps(inputs)
    res = run_sharded(in_maps)
    out = np.empty((B, L, E), dtype=np.float32)
    for c in range(N_CORES):
        b, g = divmod(c, 2)
        out[b, :, g * EC:(g + 1) * EC] = _unshard_o(res.results[c]["O"])
    return out


# revision 20
# speedup vs baseline: 1.1608x; 1.1608x over previous
"""Multi-head linear attention on Trainium2 — 8-core SPMD, batch+head sharded.

Full-tensor contract: kernel(**inputs) takes the complete Q/K/V
[4, 4096, 1024] f32 arrays, internally shards them across 8 NeuronCores
(core c -> batch c//2, heads 8*(c%2) .. 8*(c%2)+8, i.e. a contiguous
512-column slice of the embedding dim), runs one Bass kernel per core,
and reassembles the full [4, 4096, 1024] f32 output.

Per-core math (H=8 local heads, D=64, L=4096):
    phi = sigmoid(0.6053*x - 4.102)
    kv_ext[h] = phi_K[h]^T @ [V[h] | 1]     # [64, 65], f32 PSUM accum
    numden[h] = phi_Q[h] @ kv_ext[h]        # [L, 65]
    out[h]    = numden[h][:, :64] / numden[h][:, 64:65]

v2 design, balanced around four ~27-29us per-core resource floors:
  * DMA: Q/K/V inputs are fp8 e3m4 (host casts; rel err ~1.5e-2 vs the
    2e-2 gate), output fp16 -> 10.5 MB/core @ ~360 GB/s = 29us.
  * Scalar (Activation, 1.2 GHz): 4.2M sigmoids = 27.4us. It does ZERO
    DMA dispatch; loads go on the sync/vector HWDGE queues + gpsimd
    SWDGE, stores on sync/gpsimd.
  * PE: 256x (LDWEIGHTS 128 + 130 cols) = 27.5us.
  * DVE: per 2048-q piece, division = 2 batched reciprocals + 2
    multiplies using multi-bank PSUM num tiles (12 blocks in 4 banks +
    4 blocks in 2 banks, 5-dim APs) ~ 24us.

The 8 heads form 4 PAIRS, processed as a pipeline: pair g's
kv-accumulation streams while pair g-1's Q phase computes.  K and V are
merged row-wise on the host into [K_pair(128)|V_pair(128)|1|1] fp8 rows
pre-permuted into exact SBUF tile order so every DMA is a sequential
sweep with 4128 B descriptors (pairs 1-3: 2 batches of 8 tiles; pair 0:
4 batches of 4 tiles for a faster pipeline ramp).  One [128,130] matmul
per 128-row chunk (fp16 phi x fp8 V|1|1) accumulates kv AND k_sum.
phi_Q is computed per 2048-col piece so the tail pair's Q matmuls chase
the activation.  kv for a pair packs into a block-diagonal [128, 130]
fp16 operand; a 128-q matmul against it yields both heads' num|den.
"""

import numpy as np

B = 4
L = 4096
E = 1024
NH = 8            # heads per core
D = 64
W = D + 1         # head block width incl. ones/den column
EC = NH * D       # 512 embedding columns per core
P = 128
G = 4             # head pairs, stacked along rows (pipeline depth)
GC = EC // G      # 128 columns per pair
SUB = 2           # L-rows per partition line
VW = 2 * W        # 130: V|1|1 block width
KVW = GC + VW     # 258: merged K|V|1|1 staged row width
NT = L // (P * SUB)   # 16 tiles (256 L-rows) per pair
TB0 = 4           # tiles per batch, pair 0 (fast ramp)
NB0 = NT // TB0   # 4 batches for pair 0
TBS = 8           # tiles per batch, pairs 1-3 (4128 B descriptors)
NBS = NT // TBS   # 2 batches per pair
QB = 2048         # q columns per Q-phase piece
NQB = L // QB     # 2 pieces per pair
N_CORES = 8

_CACHE = {}


def _build_nc():
    from contextlib import ExitStack

    import concourse.bacc as bacc
    import concourse.bass as bass
    import concourse.mybir as mybir
    import concourse.tile as tile

    f32 = mybir.dt.float32
    f16 = mybir.dt.float16
    f8 = mybir.dt.float8e3
    SIG = mybir.ActivationFunctionType.Sigmoid

    nc = bacc.Bacc("TRN2", target_bir_lowering=False, debug=False)
    # DRAM tensors are staged by the host in exact SBUF-tile order so every
    # DMA is a fully sequential sweep with maximal descriptors.
    Q = nc.dram_tensor("Q", [EC, L], f8, kind="ExternalInput").ap()
    KV0 = nc.dram_tensor("KV0", [NB0 * P, TB0 * SUB * KVW], f8,
                         kind="ExternalInput").ap()
    KVR = nc.dram_tensor("KV", [(G - 1) * NBS * P, TBS * SUB * KVW], f8,
                         kind="ExternalInput").ap()
    O = nc.dram_tensor("O", [G * NQB * P, QB], f16, kind="ExternalOutput").ap()

    def sub_ap(t, off, dims):
        return bass.AP(tensor=t.tensor, offset=t.offset + off,
                       ap=[t.ap[0]] + dims)

    with tile.TileContext(nc) as tc, ExitStack() as ctx:
        singles = ctx.enter_context(tc.tile_pool(name="singles", bufs=1))
        ldr = ctx.enter_context(tc.tile_pool(name="ldr", bufs=6))
        ph0 = ctx.enter_context(tc.tile_pool(name="ph0", bufs=1))
        phr = ctx.enter_context(tc.tile_pool(name="phr", bufs=2))
        rcp = ctx.enter_context(tc.tile_pool(name="rcp", bufs=2))
        ob = ctx.enter_context(tc.tile_pool(name="ob", bufs=3))
        # PSUM: 2 banks kv accum + 4-bank and 2-bank num tiles = 8 banks.
        pkv = ctx.enter_context(tc.tile_pool(name="pkv", bufs=2, space="PSUM"))
        pna = ctx.enter_context(tc.tile_pool(name="pna", bufs=1, space="PSUM"))
        pnb = ctx.enter_context(tc.tile_pool(name="pnb", bufs=1, space="PSUM"))

        sig_bias = singles.tile([P, 1], f32)
        nc.vector.memset(sig_bias, -4.102)

        # Block-diagonal kv operand per head pair: rows 0:64 cols 0:65 hold
        # kv_ext of the even head, rows 64:128 cols 65:130 the odd head.
        kv_bd = singles.tile([P, G, VW], f16)
        nc.vector.memset(kv_bd, 0.0)

        # Single Q staging tile so the load is 2 coarse scalar-queue
        # dispatches (pairs 0-1, pairs 2-3) costing ~1.8us of scalar
        # sequencer time instead of 4x ~0.9us.
        q_raw = singles.tile([P, G, L], f8)
        phiq = [singles.tile([P, L], f16, tag=f"pq{g}", name=f"pq{g}")
                for g in range(G)]
        # Pair 0's four KV batches share one tile: one 4096-elem phiK
        # activation covers the whole pair (PE is idle until it anyway).
        kvt0 = singles.tile([P, NB0, TB0, SUB, KVW], f8)

        kv_tiles = {}
        kv_ps = {}

        def emit_q_load(half):
            # Two coarse dispatches (pairs 0-1, 2-3) keep the scalar
            # sequencer cost low; they queue BEHIND kvr(1,0) on the
            # scalar HWDGE queue so the ramp-critical KV batch wins.
            nc.scalar.dma_start(
                out=q_raw[:, 2 * half:2 * half + 2, :],
                in_=Q[2 * half * P:(2 * half + 2) * P, :].rearrange(
                    "(k p) l -> p k l", k=2))

        def emit_kv_dispatch0(b):
            # b0-b2 on sync (fast-starting HWDGE); only b3 on gpsimd,
            # whose SWDGE queue delivers ~4us later than HWDGE.
            eng = nc.gpsimd if b == 3 else nc.sync
            eng.dma_start(
                out=kvt0[:, b, :, :, :],
                in_=KV0[b * P:(b + 1) * P, :].rearrange(
                    "p (t s e) -> p t s e", t=TB0, s=SUB))

        def emit_kv_dispatch_r(g, ib):
            r0 = ((g - 1) * NBS + ib) * P
            kvt = ldr.tile([P, TBS, SUB, KVW], f8, tag="kvtr", name="kvtr")
            # (1,0) rides the otherwise-idle scalar queue ahead of the Q
            # stream so pair-1 compute starts right after pair 0.
            if (g, ib) == (1, 0):
                eng = nc.scalar
            else:
                eng = nc.sync if ib == 0 else nc.gpsimd
            eng.dma_start(
                out=kvt,
                in_=KVR[r0:r0 + P, :].rearrange(
                    "p (t s e) -> p t s e", t=TBS, s=SUB))
            kv_tiles[(g, ib)] = kvt

        def emit_kv_compute0(b):
            # Per-batch activation so compute chases each landing batch.
            phiK = ph0.tile([P, TB0, SUB, GC], f16, tag="phiK0",
                            name="phiK0", bufs=2)
            nc.scalar.activation(
                out=phiK, in_=kvt0[:, b, :, :, 0:GC], func=SIG,
                bias=sig_bias, scale=0.6053)
            for t in range(TB0):
                for s in range(SUB):
                    nc.tensor.matmul(
                        out=kv_ps[0][:, 0:VW],
                        lhsT=phiK[:, t, s, :],
                        rhs=kvt0[:, b, t, s, GC:KVW],
                        start=(b == 0 and t == 0 and s == 0),
                        stop=(b == NB0 - 1 and t == TB0 - 1
                              and s == SUB - 1))

        def emit_kv_compute(g, ib):
            kvt = kv_tiles.pop((g, ib))
            phiK = phr.tile([P, TBS, SUB, GC], f16, tag="phiKr", name="phiK")
            nc.scalar.activation(
                out=phiK, in_=kvt[:, :, :, 0:GC], func=SIG, bias=sig_bias,
                scale=0.6053)
            for t in range(TBS):
                for s in range(SUB):
                    nc.tensor.matmul(
                        out=kv_ps[g][:, 0:VW],
                        lhsT=phiK[:, t, s, :],
                        rhs=kvt[:, t, s, GC:KVW],
                        start=(ib == 0 and t == 0 and s == 0),
                        stop=(ib == NBS - 1 and t == TBS - 1
                              and s == SUB - 1))

        def emit_phiq(g, qb=None):
            # qb=None: whole pair in one activation (pairs 0-2); pair 3
            # stays split per piece so its Q matmuls chase the activation.
            sl = slice(0, L) if qb is None else slice(qb * QB, (qb + 1) * QB)
            nc.scalar.activation(
                out=phiq[g][:, sl], in_=q_raw[:, g, sl],
                func=SIG, bias=sig_bias, scale=0.6053)

        def emit_kv_finish(g):
            """Pack the pair's kv PSUM tile into the block-diag operand."""
            ps = kv_ps.pop(g)
            nc.vector.tensor_copy(out=kv_bd[0:D, g, 0:D], in_=ps[0:D, 0:D])
            nc.vector.tensor_copy(
                out=kv_bd[0:D, g, D:W], in_=ps[0:D, 2 * D:2 * D + 1])
            nc.vector.tensor_copy(
                out=kv_bd[D:P, g, W:W + D], in_=ps[D:P, D:2 * D])
            nc.vector.tensor_copy(
                out=kv_bd[D:P, g, W + D:VW], in_=ps[D:P, 2 * D:2 * D + 1])

        def emit_q_piece(g, qb, split_store=False):
            """16 q-block matmuls + batched recip/mult + store.

            num blocks live 3-per-bank in a 4-bank (12 blocks) and a
            2-bank (4 blocks, 2 per bank) PSUM tile; one reciprocal and
            one 5-dim-AP multiply covers each tile."""
            pi = g * NQB + qb
            q0 = qb * QB
            orow = pi * P
            out_t = ob.tile([P, QB], f16, tag="outt", name="out_t")

            numA = pna.tile([P, 2048], f32, tag="numA", name="numA")
            for blk in range(12):
                k, j = divmod(blk, 3)
                nc.tensor.matmul(
                    out=sub_ap(numA, 512 * k + 130 * j, [[1, VW]]),
                    lhsT=phiq[g][:, q0 + blk * P:q0 + (blk + 1) * P],
                    rhs=kv_bd[:, g, :], start=True, stop=True)
            rA = rcp.tile([P, 4, 3, 2], f32, tag="rA", name="rA")
            nc.vector.reciprocal(
                out=rA, in_=sub_ap(numA, D, [[512, 4], [130, 3], [W, 2]]))
            nc.vector.tensor_tensor(
                out=sub_ap(out_t, 0, [[384, 4], [128, 3], [64, 2], [1, 64]]),
                in0=sub_ap(numA, 0, [[512, 4], [130, 3], [W, 2], [1, D]]),
                in1=sub_ap(rA, 0, [[6, 4], [2, 3], [1, 2], [0, D]]),
                op=mybir.AluOpType.mult)
            if split_store:
                engA = nc.sync if pi % 2 == 0 else nc.gpsimd
                engA.dma_start(out=O[orow:orow + P, 0:1536],
                               in_=out_t[:, 0:1536])

            numB = pnb.tile([P, 1024], f32, tag="numB", name="numB")
            for blk in range(12, 16):
                k, j = divmod(blk - 12, 2)
                nc.tensor.matmul(
                    out=sub_ap(numB, 512 * k + 130 * j, [[1, VW]]),
                    lhsT=phiq[g][:, q0 + blk * P:q0 + (blk + 1) * P],
                    rhs=kv_bd[:, g, :], start=True, stop=True)
            rB = rcp.tile([P, 2, 2, 2], f32, tag="rB", name="rB")
            nc.vector.reciprocal(
                out=rB, in_=sub_ap(numB, D, [[512, 2], [130, 2], [W, 2]]))
            nc.vector.tensor_tensor(
                out=sub_ap(out_t, 1536, [[256, 2], [128, 2], [64, 2], [1, 64]]),
                in0=sub_ap(numB, 0, [[512, 2], [130, 2], [W, 2], [1, D]]),
                in1=sub_ap(rB, 0, [[4, 2], [2, 2], [1, 2], [0, D]]),
                op=mybir.AluOpType.mult)
            if split_store:
                engB = nc.gpsimd if pi % 2 == 0 else nc.sync
                engB.dma_start(out=O[orow:orow + P, 1536:QB],
                               in_=out_t[:, 1536:QB])
            else:
                eng = nc.sync if pi % 2 == 0 else nc.gpsimd
                eng.dma_start(out=O[orow:orow + P, :], in_=out_t)

        # Dummy sigmoid on the bias column: pulls the framework's
        # activation-table load to the head of the scalar stream where it
        # overlaps the idle ramp (the load is async; only the first
        # dependent activation waits on it).
        warm = singles.tile([P, 1], f32)
        nc.scalar.activation(out=warm, in_=sig_bias, func=SIG,
                             bias=sig_bias, scale=1.0)

        # ---- load dispatches.  Per-engine streams:
        #   scalar: kvr(1,0), QA (pairs 0-1), QB (pairs 2-3)
        #   sync:   kv0 b0/b2, kvr(2,0), kvr(3,0), even stores
        #   gpsimd: kv0 b1/b3, kvr(1,1), kvr(2,1), kvr(3,1), odd stores
        emit_kv_dispatch_r(1, 0)
        emit_q_load(0)
        emit_q_load(1)
        for b in range(NB0):
            emit_kv_dispatch0(b)
        emit_kv_dispatch_r(1, 1)
        for g in range(2, G):
            for ib in range(NBS):
                emit_kv_dispatch_r(g, ib)

        # ---- software-pipelined compute: pair g's kv streams while
        # pair g-1's Q phase computes.  phiQ(g-1) interleaves with pair
        # g's phiK so the scalar stream never starves the PE. ----
        kv_ps[0] = pkv.tile([P, 512], f32, tag="kv", name="kv")
        for b in range(NB0):
            emit_kv_compute0(b)
        emit_kv_finish(0)
        for g in range(1, G):
            kv_ps[g] = pkv.tile([P, 512], f32, tag="kv", name="kv")
            emit_kv_compute(g, 0)
            emit_phiq(g - 1, 0)
            emit_q_piece(g - 1, 0)
            emit_kv_compute(g, 1)
            emit_phiq(g - 1, 1)
            emit_q_piece(g - 1, 1)
            emit_kv_finish(g)
        emit_phiq(G - 1, 0)
        emit_q_piece(G - 1, 0, split_store=True)
        emit_phiq(G - 1, 1)
        emit_q_piece(G - 1, 1, split_store=True)

    nc.compile()
    return nc


def _get_nc():
    if "nc" not in _CACHE:
        _CACHE["nc"] = _build_nc()
    return _CACHE["nc"]


def _f8():
    import ml_dtypes
    return ml_dtypes.float8_e3m4


def _shard_q(arr):
    """Full [B, L, E] f32 -> per-core transposed [512, L] fp8 slices."""
    f8 = _f8()
    out = []
    for c in range(N_CORES):
        b, g = divmod(c, 2)
        out.append(np.ascontiguousarray(
            arr[b, :, g * EC:(g + 1) * EC].T.astype(f8)))
    return out


def _shard_kv(karr, varr):
    """Full K/V [B, L, E] f32 -> per-core (KV0, KVR) fp8 staging: merged
    [K_pair(128) | V_pair(128) | 1 | 1] rows pre-permuted into SBUF tile
    order [ib][p][t][s][e] so every batch DMA is one sequential sweep."""
    f8 = _f8()
    out = []
    for c in range(N_CORES):
        b, g = divmod(c, 2)
        ksl = karr[b, :, g * EC:(g + 1) * EC]
        vsl = varr[b, :, g * EC:(g + 1) * EC]
        st = np.ones((G, L, KVW), dtype=f8)
        for pg in range(G):
            st[pg, :, 0:GC] = ksl[:, pg * GC:(pg + 1) * GC].astype(f8)
            st[pg, :, GC:GC + P] = vsl[:, pg * P:(pg + 1) * P].astype(f8)
        # row (ib, t, p, s) -> staged position (ib, p, t, s)
        kv0 = st[0].reshape(NB0, TB0, P, SUB, KVW).transpose(
            0, 2, 1, 3, 4).reshape(NB0 * P, TB0 * SUB * KVW)
        kvr = st[1:].reshape(G - 1, NBS, TBS, P, SUB, KVW).transpose(
            0, 1, 3, 2, 4, 5).reshape((G - 1) * NBS * P, TBS * SUB * KVW)
        out.append((np.ascontiguousarray(kv0), np.ascontiguousarray(kvr)))
    return out


def _unshard_o(o):
    """Per-core [4*2*128, 2048] fp16 (piece-major) -> [L, EC] f32 slice."""
    blocks = np.asarray(o).reshape(G, NQB, P, QB // P, P)  # [pg, qb, p, qk, e]
    # q = qb*QB + qk*P + p
    perm = blocks.transpose(0, 1, 3, 2, 4).reshape(G, L, P)
    return np.concatenate(list(perm), axis=1).astype(np.float32)


def make_in_maps(inputs):
    Q = np.asarray(inputs["Q"], dtype=np.float32)
    K = np.asarray(inputs["K"], dtype=np.float32)
    V = np.asarray(inputs["V"], dtype=np.float32)
    qs, kvs = _shard_q(Q), _shard_kv(K, V)
    return [{"Q": qs[c], "KV0": kvs[c][0], "KV": kvs[c][1]}
            for c in range(N_CORES)]


def run_sharded(in_maps, trace=False, trace_cores=None):
    from concourse.bass_utils import run_bass_kernel_spmd

    nc = _get_nc()
    kwargs = {}
    if trace:
        kwargs = dict(trace=True, trace_cores=trace_cores or [0])
    return run_bass_kernel_spmd(nc, in_maps, core_ids=list(range(N_CORES)),
                                **kwargs)


def kernel(**inputs):
    in_maps = make_in_maps(inputs)
    res = run_sharded(in_maps)
    out = np.empty((B, L, E), dtype=np.float32)
    for c in range(N_CORES):
        b, g = divmod(c, 2)
        out[b, :, g * EC:(g + 1) * EC] = _unshard_o(res.results[c]["O"])
    return out


# revision 22
# speedup vs baseline: 1.2047x; 1.0378x over previous
"""Multi-head linear attention on Trainium2 — 8-core SPMD, batch+head sharded.

Full-tensor contract: kernel(**inputs) takes the complete Q/K/V
[4, 4096, 1024] f32 arrays, internally shards them across 8 NeuronCores
(core c -> batch c//2, heads 8*(c%2) .. 8*(c%2)+8, i.e. a contiguous
512-column slice of the embedding dim), runs one Bass kernel per core,
and reassembles the full [4, 4096, 1024] f32 output.

Per-core math (H=8 local heads, D=64, L=4096):
    phi = sigmoid(0.6053*x - 4.102)
    kv_ext[h] = phi_K[h]^T @ [V[h] | 1]     # [64, 65], f32 PSUM accum
    numden[h] = phi_Q[h] @ kv_ext[h]        # [L, 65]
    out[h]    = numden[h][:, :64] / numden[h][:, 64:65]

v2 design, balanced around four ~27-29us per-core resource floors:
  * DMA: Q/K/V inputs are fp8 e3m4 (host casts; rel err ~1.5e-2 vs the
    2e-2 gate), output fp16 -> 10.5 MB/core @ ~360 GB/s = 29us.
  * Scalar (Activation, 1.2 GHz): 4.2M sigmoids = 27.4us. It does ZERO
    DMA dispatch; loads go on the sync/vector HWDGE queues + gpsimd
    SWDGE, stores on sync/gpsimd.
  * PE: 256x (LDWEIGHTS 128 + 130 cols) = 27.5us.
  * DVE: per 2048-q piece, division = 2 batched reciprocals + 2
    multiplies using multi-bank PSUM num tiles (12 blocks in 4 banks +
    4 blocks in 2 banks, 5-dim APs) ~ 24us.

The 8 heads form 4 PAIRS, processed as a pipeline: pair g's
kv-accumulation streams while pair g-1's Q phase computes.  K and V are
merged row-wise on the host into [K_pair(128)|V_pair(128)|1|1] fp8 rows
pre-permuted into exact SBUF tile order so every DMA is a sequential
sweep with 4128 B descriptors (pairs 1-3: 2 batches of 8 tiles; pair 0:
4 batches of 4 tiles for a faster pipeline ramp).  One [128,130] matmul
per 128-row chunk (fp16 phi x fp8 V|1|1) accumulates kv AND k_sum.
phi_Q is computed per 2048-col piece so the tail pair's Q matmuls chase
the activation.  kv for a pair packs into a block-diagonal [128, 130]
fp16 operand; a 128-q matmul against it yields both heads' num|den.
"""

import numpy as np

B = 4
L = 4096
E = 1024
NH = 8            # heads per core
D = 64
W = D + 1         # head block width incl. ones/den column
EC = NH * D       # 512 embedding columns per core
P = 128
G = 4             # head pairs, stacked along rows (pipeline depth)
GC = EC // G      # 128 columns per pair
SUB = 2           # L-rows per partition line
VW = 2 * W        # 130: V|1|1 block width
KVW = GC + VW     # 258: merged K|V|1|1 staged row width
NT = L // (P * SUB)   # 16 tiles (256 L-rows) per pair
TB0 = 4           # tiles per batch, pair 0 (fast ramp)
NB0 = NT // TB0   # 4 batches for pair 0
TBS = 8           # tiles per batch, pairs 1-3 (4128 B descriptors)
NBS = NT // TBS   # 2 batches per pair
QB = 2048         # q columns per Q-phase piece
NQB = L // QB     # 2 pieces per pair
N_CORES = 8

_CACHE = {}


def _build_nc():
    from contextlib import ExitStack

    import concourse.bacc as bacc
    import concourse.bass as bass
    import concourse.mybir as mybir
    import concourse.tile as tile

    f32 = mybir.dt.float32
    f16 = mybir.dt.float16
    f8 = mybir.dt.float8e3
    SIG = mybir.ActivationFunctionType.Sigmoid

    nc = bacc.Bacc("TRN2", target_bir_lowering=False, debug=False)
    # DRAM tensors are staged by the host in exact SBUF-tile order so every
    # DMA is a fully sequential sweep with maximal descriptors.
    Q = nc.dram_tensor("Q", [EC, L], f8, kind="ExternalInput").ap()
    KV0 = nc.dram_tensor("KV0", [NB0 * P, TB0 * SUB * KVW], f8,
                         kind="ExternalInput").ap()
    KVR = nc.dram_tensor("KV", [(G - 1) * NBS * P, TBS * SUB * KVW], f8,
                         kind="ExternalInput").ap()
    O = nc.dram_tensor("O", [G * NQB * P, QB], f16, kind="ExternalOutput").ap()

    def sub_ap(t, off, dims):
        return bass.AP(tensor=t.tensor, offset=t.offset + off,
                       ap=[t.ap[0]] + dims)

    with tile.TileContext(nc) as tc, ExitStack() as ctx:
        singles = ctx.enter_context(tc.tile_pool(name="singles", bufs=1))
        ldr = ctx.enter_context(tc.tile_pool(name="ldr", bufs=6))
        ph0 = ctx.enter_context(tc.tile_pool(name="ph0", bufs=1))
        phr = ctx.enter_context(tc.tile_pool(name="phr", bufs=2))
        rcp = ctx.enter_context(tc.tile_pool(name="rcp", bufs=2))
        ob = ctx.enter_context(tc.tile_pool(name="ob", bufs=3))
        # PSUM: 2 banks kv accum + 2x2-bank numA (double-buffered) +
        # 2-bank numB = 8 banks.
        pkv = ctx.enter_context(tc.tile_pool(name="pkv", bufs=2, space="PSUM"))
        pna = ctx.enter_context(tc.tile_pool(name="pna", bufs=2, space="PSUM"))
        pnb = ctx.enter_context(tc.tile_pool(name="pnb", bufs=1, space="PSUM"))

        sig_bias = singles.tile([P, 1], f32)
        nc.vector.memset(sig_bias, -4.102)

        # Block-diagonal kv operand per head pair: rows 0:64 cols 0:65 hold
        # kv_ext of the even head, rows 64:128 cols 65:130 the odd head.
        kv_bd = singles.tile([P, G, VW], f16)
        nc.vector.memset(kv_bd, 0.0)

        # Single Q staging tile so the load is 2 coarse scalar-queue
        # dispatches (pairs 0-1, pairs 2-3) costing ~1.8us of scalar
        # sequencer time instead of 4x ~0.9us.
        q_raw = singles.tile([P, G, L], f8)
        phiq = [singles.tile([P, L], f16, tag=f"pq{g}", name=f"pq{g}")
                for g in range(G)]
        # Pair 0's four KV batches share one tile: one 4096-elem phiK
        # activation covers the whole pair (PE is idle until it anyway).
        kvt0 = singles.tile([P, NB0, TB0, SUB, KVW], f8)

        kv_tiles = {}
        kv_ps = {}

        def emit_q_load(half):
            # Two coarse dispatches (pairs 0-1, 2-3) keep the scalar
            # sequencer cost low; they queue BEHIND kvr(1,0) on the
            # scalar HWDGE queue so the ramp-critical KV batch wins.
            nc.scalar.dma_start(
                out=q_raw[:, 2 * half:2 * half + 2, :],
                in_=Q[2 * half * P:(2 * half + 2) * P, :].rearrange(
                    "(k p) l -> p k l", k=2))

        def emit_kv_dispatch0(b):
            # b0-b2 on sync (fast-starting HWDGE); only b3 on gpsimd,
            # whose SWDGE queue delivers ~4us later than HWDGE.
            eng = nc.gpsimd if b == 3 else nc.sync
            eng.dma_start(
                out=kvt0[:, b, :, :, :],
                in_=KV0[b * P:(b + 1) * P, :].rearrange(
                    "p (t s e) -> p t s e", t=TB0, s=SUB))

        def emit_kv_dispatch_r(g, ib):
            r0 = ((g - 1) * NBS + ib) * P
            kvt = ldr.tile([P, TBS, SUB, KVW], f8, tag="kvtr", name="kvtr")
            # (1,0) rides the otherwise-idle scalar queue ahead of the Q
            # stream so pair-1 compute starts right after pair 0.
            if (g, ib) == (1, 0):
                eng = nc.scalar
            else:
                eng = nc.sync if ib == 0 else nc.gpsimd
            eng.dma_start(
                out=kvt,
                in_=KVR[r0:r0 + P, :].rearrange(
                    "p (t s e) -> p t s e", t=TBS, s=SUB))
            kv_tiles[(g, ib)] = kvt

        def emit_kv_compute0(b):
            # Per-batch activation so compute chases each landing batch.
            phiK = ph0.tile([P, TB0, SUB, GC], f16, tag="phiK0",
                            name="phiK0", bufs=2)
            nc.scalar.activation(
                out=phiK, in_=kvt0[:, b, :, :, 0:GC], func=SIG,
                bias=sig_bias, scale=0.6053)
            for t in range(TB0):
                for s in range(SUB):
                    nc.tensor.matmul(
                        out=kv_ps[0][:, 0:VW],
                        lhsT=phiK[:, t, s, :],
                        rhs=kvt0[:, b, t, s, GC:KVW],
                        start=(b == 0 and t == 0 and s == 0),
                        stop=(b == NB0 - 1 and t == TB0 - 1
                              and s == SUB - 1))

        def emit_kv_compute(g, ib):
            kvt = kv_tiles.pop((g, ib))
            phiK = phr.tile([P, TBS, SUB, GC], f16, tag="phiKr", name="phiK")
            nc.scalar.activation(
                out=phiK, in_=kvt[:, :, :, 0:GC], func=SIG, bias=sig_bias,
                scale=0.6053)
            for t in range(TBS):
                for s in range(SUB):
                    nc.tensor.matmul(
                        out=kv_ps[g][:, 0:VW],
                        lhsT=phiK[:, t, s, :],
                        rhs=kvt[:, t, s, GC:KVW],
                        start=(ib == 0 and t == 0 and s == 0),
                        stop=(ib == NBS - 1 and t == TBS - 1
                              and s == SUB - 1))

        def emit_phiq(g, qb=None):
            # qb=None: whole pair in one activation (pairs 0-2); pair 3
            # stays split per piece so its Q matmuls chase the activation.
            sl = slice(0, L) if qb is None else slice(qb * QB, (qb + 1) * QB)
            nc.scalar.activation(
                out=phiq[g][:, sl], in_=q_raw[:, g, sl],
                func=SIG, bias=sig_bias, scale=0.6053)

        def emit_kv_finish(g):
            """Pack the pair's kv PSUM tile into the block-diag operand."""
            ps = kv_ps.pop(g)
            nc.vector.tensor_copy(out=kv_bd[0:D, g, 0:D], in_=ps[0:D, 0:D])
            nc.vector.tensor_copy(
                out=kv_bd[0:D, g, D:W], in_=ps[0:D, 2 * D:2 * D + 1])
            nc.vector.tensor_copy(
                out=kv_bd[D:P, g, W:W + D], in_=ps[D:P, D:2 * D])
            nc.vector.tensor_copy(
                out=kv_bd[D:P, g, W + D:VW], in_=ps[D:P, 2 * D:2 * D + 1])

        def emit_q_piece(g, qb, split_store=False):
            """16 q-block matmuls + batched recip/mult + store.

            num blocks live 3-per-bank in two double-buffered 2-bank
            tiles (6 blocks each) and one 2-bank tile (4 blocks, 2 per
            bank); one reciprocal and one 5-dim-AP multiply per group.
            Double-buffered numA means the next piece's matmuls never
            wait on this piece's multiplies (tail-critical)."""
            pi = g * NQB + qb
            q0 = qb * QB
            orow = pi * P
            out_t = ob.tile([P, QB], f16, tag="outt", name="out_t")
            engs = ([nc.sync, nc.gpsimd] if pi % 2 == 0
                    else [nc.gpsimd, nc.sync])

            def group(blk0, nk, num, r, c0):
                for i in range(nk * 3):
                    k, j = divmod(i, 3)
                    nc.tensor.matmul(
                        out=sub_ap(num, 512 * k + 130 * j, [[1, VW]]),
                        lhsT=phiq[g][:, q0 + (blk0 + i) * P:
                                     q0 + (blk0 + i + 1) * P],
                        rhs=kv_bd[:, g, :], start=True, stop=True)
                nc.vector.reciprocal(
                    out=r, in_=sub_ap(num, D, [[512, nk], [130, 3], [W, 2]]))
                nc.vector.tensor_tensor(
                    out=sub_ap(out_t, c0,
                               [[384, nk], [128, 3], [64, 2], [1, 64]]),
                    in0=sub_ap(num, 0, [[512, nk], [130, 3], [W, 2], [1, D]]),
                    in1=sub_ap(r, 0, [[6, nk], [2, 3], [1, 2], [0, D]]),
                    op=mybir.AluOpType.mult)
                if split_store:
                    engs[(c0 // 768) % 2].dma_start(
                        out=O[orow:orow + P, c0:c0 + nk * 384],
                        in_=out_t[:, c0:c0 + nk * 384])

            for h in range(2):
                numA = pna.tile([P, 1024], f32, tag="numA", name="numA")
                rA = rcp.tile([P, 2, 3, 2], f32, tag="rA", name="rA")
                group(6 * h, 2, numA, rA, 768 * h)

            numB = pnb.tile([P, 1024], f32, tag="numB", name="numB")
            for blk in range(12, 16):
                k, j = divmod(blk - 12, 2)
                nc.tensor.matmul(
                    out=sub_ap(numB, 512 * k + 130 * j, [[1, VW]]),
                    lhsT=phiq[g][:, q0 + blk * P:q0 + (blk + 1) * P],
                    rhs=kv_bd[:, g, :], start=True, stop=True)
            rB = rcp.tile([P, 2, 2, 2], f32, tag="rB", name="rB")
            nc.vector.reciprocal(
                out=rB, in_=sub_ap(numB, D, [[512, 2], [130, 2], [W, 2]]))
            nc.vector.tensor_tensor(
                out=sub_ap(out_t, 1536, [[256, 2], [128, 2], [64, 2], [1, 64]]),
                in0=sub_ap(numB, 0, [[512, 2], [130, 2], [W, 2], [1, D]]),
                in1=sub_ap(rB, 0, [[4, 2], [2, 2], [1, 2], [0, D]]),
                op=mybir.AluOpType.mult)
            if split_store:
                engs[0].dma_start(out=O[orow:orow + P, 1536:QB],
                                  in_=out_t[:, 1536:QB])
            else:
                engs[0].dma_start(out=O[orow:orow + P, :], in_=out_t)

        # Dummy sigmoid on the bias column: pulls the framework's
        # activation-table load to the head of the scalar stream where it
        # overlaps the idle ramp (the load is async; only the first
        # dependent activation waits on it).
        warm = singles.tile([P, 1], f32)
        nc.scalar.activation(out=warm, in_=sig_bias, func=SIG,
                             bias=sig_bias, scale=1.0)

        # ---- load dispatches.  Per-engine streams:
        #   scalar: kvr(1,0), QA (pairs 0-1), QB (pairs 2-3)
        #   sync:   kv0 b0/b2, kvr(2,0), kvr(3,0), even stores
        #   gpsimd: kv0 b1/b3, kvr(1,1), kvr(2,1), kvr(3,1), odd stores
        emit_kv_dispatch_r(1, 0)
        emit_q_load(0)
        emit_q_load(1)
        for b in range(NB0):
            emit_kv_dispatch0(b)
        emit_kv_dispatch_r(1, 1)
        for g in range(2, G):
            for ib in range(NBS):
                emit_kv_dispatch_r(g, ib)

        # ---- software-pipelined compute: pair g's kv streams while
        # pair g-1's Q phase computes.  phiQ(g-1) interleaves with pair
        # g's phiK so the scalar stream never starves the PE. ----
        kv_ps[0] = pkv.tile([P, 512], f32, tag="kv", name="kv")
        for b in range(NB0):
            emit_kv_compute0(b)
        emit_kv_finish(0)
        for g in range(1, G):
            kv_ps[g] = pkv.tile([P, 512], f32, tag="kv", name="kv")
            emit_kv_compute(g, 0)
            emit_phiq(g - 1, 0)
            emit_q_piece(g - 1, 0)
            emit_kv_compute(g, 1)
            emit_phiq(g - 1, 1)
            emit_q_piece(g - 1, 1)
            emit_kv_finish(g)
        emit_phiq(G - 1, 0)
        emit_q_piece(G - 1, 0, split_store=True)
        emit_phiq(G - 1, 1)
        emit_q_piece(G - 1, 1, split_store=True)

    nc.compile()
    return nc


def _get_nc():
    if "nc" not in _CACHE:
        _CACHE["nc"] = _build_nc()
    return _CACHE["nc"]


def _f8():
    import ml_dtypes
    return ml_dtypes.float8_e3m4


def _shard_q(arr):
    """Full [B, L, E] f32 -> per-core transposed [512, L] fp8 slices."""
    f8 = _f8()
    out = []
    for c in range(N_CORES):
        b, g = divmod(c, 2)
        out.append(np.ascontiguousarray(
            arr[b, :, g * EC:(g + 1) * EC].T.astype(f8)))
    return out


def _shard_kv(karr, varr):
    """Full K/V [B, L, E] f32 -> per-core (KV0, KVR) fp8 staging: merged
    [K_pair(128) | V_pair(128) | 1 | 1] rows pre-permuted into SBUF tile
    order [ib][p][t][s][e] so every batch DMA is one sequential sweep."""
    f8 = _f8()
    out = []
    for c in range(N_CORES):
        b, g = divmod(c, 2)
        ksl = karr[b, :, g * EC:(g + 1) * EC]
        vsl = varr[b, :, g * EC:(g + 1) * EC]
        st = np.ones((G, L, KVW), dtype=f8)
        for pg in range(G):
            st[pg, :, 0:GC] = ksl[:, pg * GC:(pg + 1) * GC].astype(f8)
            st[pg, :, GC:GC + P] = vsl[:, pg * P:(pg + 1) * P].astype(f8)
        # row (ib, t, p, s) -> staged position (ib, p, t, s)
        kv0 = st[0].reshape(NB0, TB0, P, SUB, KVW).transpose(
            0, 2, 1, 3, 4).reshape(NB0 * P, TB0 * SUB * KVW)
        kvr = st[1:].reshape(G - 1, NBS, TBS, P, SUB, KVW).transpose(
            0, 1, 3, 2, 4, 5).reshape((G - 1) * NBS * P, TBS * SUB * KVW)
        out.append((np.ascontiguousarray(kv0), np.ascontiguousarray(kvr)))
    return out


def _unshard_o(o):
    """Per-core [4*2*128, 2048] fp16 (piece-major) -> [L, EC] f32 slice."""
    blocks = np.asarray(o).reshape(G, NQB, P, QB // P, P)  # [pg, qb, p, qk, e]
    # q = qb*QB + qk*P + p
    perm = blocks.transpose(0, 1, 3, 2, 4).reshape(G, L, P)
    return np.concatenate(list(perm), axis=1).astype(np.float32)


def make_in_maps(inputs):
    Q = np.asarray(inputs["Q"], dtype=np.float32)
    K = np.asarray(inputs["K"], dtype=np.float32)
    V = np.asarray(inputs["V"], dtype=np.float32)
    qs, kvs = _shard_q(Q), _shard_kv(K, V)
    return [{"Q": qs[c], "KV0": kvs[c][0], "KV": kvs[c][1]}
            for c in range(N_CORES)]


def run_sharded(in_maps, trace=False, trace_cores=None):
    from concourse.bass_utils import run_bass_kernel_spmd

    nc = _get_nc()
    kwargs = {}
    if trace:
        kwargs = dict(trace=True, trace_cores=trace_cores or [0])
    return run_bass_kernel_spmd(nc, in_maps, core_ids=list(range(N_CORES)),
                                **kwargs)


def kernel(**inputs):
    in_maps = make_in_maps(inputs)
    res = run_sharded(in_maps)
    out = np.empty((B, L, E), dtype=np.float32)
    for c in range(N_CORES):
        b, g = divmod(c, 2)
        out[b, :, g * EC:(g + 1) * EC] = _unshard_o(res.results[c]["O"])
    return out


# revision 23
# speedup vs baseline: 1.2419x; 1.0308x over previous
"""Multi-head linear attention on Trainium2 — 8-core SPMD, batch+head sharded.

Full-tensor contract: kernel(**inputs) takes the complete Q/K/V
[4, 4096, 1024] f32 arrays, internally shards them across 8 NeuronCores
(core c -> batch c//2, heads 8*(c%2) .. 8*(c%2)+8, i.e. a contiguous
512-column slice of the embedding dim), runs one Bass kernel per core,
and reassembles the full [4, 4096, 1024] f32 output.

Per-core math (H=8 local heads, D=64, L=4096):
    phi = sigmoid(0.6053*x - 4.102)
    kv_ext[h] = phi_K[h]^T @ [V[h] | 1]     # [64, 65], f32 PSUM accum
    numden[h] = phi_Q[h] @ kv_ext[h]        # [L, 65]
    out[h]    = numden[h][:, :64] / numden[h][:, 64:65]

Design, balanced around four ~27-30us per-core resource floors:
  * DMA: Q/K/V inputs are fp8 e3m4 (host casts; rel err ~1.5e-2 vs the
    2e-2 gate), output fp16 -> 10.5 MB/core @ ~360 GB/s = 29us.
  * Scalar (Activation, 1.2 GHz): 4.2M sigmoids = ~32us measured (the
    chip's DVFS throttle keeps the effective clock ~15% down).  This is
    THE critical chain, so scalar does almost no DMA dispatch.
  * PE: 256x (LDWEIGHTS 128 + 130 cols) = ~27us.
  * DVE: per-piece division = batched reciprocals + 5-dim-AP multiplies
    over multi-bank PSUM num tiles = ~28us.

The 8 heads form 4 PAIRS, processed as a pipeline: pair g's
kv-accumulation streams while pair g-1's Q phase computes.  K and V are
merged row-wise on the host into [K_pair(128)|V_pair(128)|1|1] fp8 rows
pre-permuted into exact SBUF tile order so every DMA is a sequential
sweep with fat descriptors (pairs 1-3: 2 batches of 8 tiles, 4128 B
rows; pair 0: 4 batches of 4 tiles for a faster pipeline ramp).  One
[128,130] matmul per 128-row chunk (fp16 phi x fp8 V|1|1) accumulates
kv AND k_sum via the baked-in ones columns.

Queue plan (DMA dispatch instructions cost ~0.8us of engine time, and
per-dispatch arbitration means a big dispatch can monopolize the DMA
engines, so placement matters): sync HWDGE carries pair-0 b0-b2 +
kvr(2,0)/(3,0); gpsimd SWDGE (starts ~4us later) carries b3 +
kvr(*,1); the scalar queue carries kvr(1,0) then two coarse Q-load
dispatches; O-piece stores alternate sync/gpsimd, dispatched after
their data is complete so the (idle) engines just fire them.

phi_Q(g) is computed per 2048-col piece, interleaved one pair late in
the scalar stream so phi_K(g) is never delayed; the framework's
list-scheduler further reorders activations by data readiness.  A
dummy sigmoid at stream head prefetches the activation table (the
table load is async).  kv for a pair packs into a block-diagonal
[128, 130] fp16 operand; a 128-q matmul against it yields both heads'
num|den.  Division per piece: 16 q-blocks as groups of 6+6+4 in
2-bank PSUM tiles (3 blocks per bank; numA double-buffered so the next
piece's matmuls never wait on this piece's multiplies -- tail
critical), one reciprocal + one 5-dim-AP broadcast multiply per group;
the final pair's pieces store per-group to drain the tail early.
"""

import numpy as np

B = 4
L = 4096
E = 1024
NH = 8            # heads per core
D = 64
W = D + 1         # head block width incl. ones/den column
EC = NH * D       # 512 embedding columns per core
P = 128
G = 4             # head pairs, stacked along rows (pipeline depth)
GC = EC // G      # 128 columns per pair
SUB = 2           # L-rows per partition line
VW = 2 * W        # 130: V|1|1 block width
KVW = GC + VW     # 258: merged K|V|1|1 staged row width
NT = L // (P * SUB)   # 16 tiles (256 L-rows) per pair
TB0 = 4           # tiles per batch, pair 0 (fast ramp)
NB0 = NT // TB0   # 4 batches for pair 0
TBS = 8           # tiles per batch, pairs 1-3 (4128 B descriptors)
NBS = NT // TBS   # 2 batches per pair
QB = 2048         # q columns per Q-phase piece
NQB = L // QB     # 2 pieces per pair
N_CORES = 8

_CACHE = {}


def _build_nc():
    from contextlib import ExitStack

    import concourse.bacc as bacc
    import concourse.bass as bass
    import concourse.mybir as mybir
    import concourse.tile as tile

    f32 = mybir.dt.float32
    f16 = mybir.dt.float16
    f8 = mybir.dt.float8e3
    SIG = mybir.ActivationFunctionType.Sigmoid

    nc = bacc.Bacc("TRN2", target_bir_lowering=False, debug=False)
    # DRAM tensors are staged by the host in exact SBUF-tile order so every
    # DMA is a fully sequential sweep with maximal descriptors.
    Q = nc.dram_tensor("Q", [EC, L], f8, kind="ExternalInput").ap()
    KV0 = nc.dram_tensor("KV0", [NB0 * P, TB0 * SUB * KVW], f8,
                         kind="ExternalInput").ap()
    KVR = nc.dram_tensor("KV", [(G - 1) * NBS * P, TBS * SUB * KVW], f8,
                         kind="ExternalInput").ap()
    O = nc.dram_tensor("O", [G * NQB * P, QB], f16, kind="ExternalOutput").ap()

    def sub_ap(t, off, dims):
        return bass.AP(tensor=t.tensor, offset=t.offset + off,
                       ap=[t.ap[0]] + dims)

    with tile.TileContext(nc) as tc, ExitStack() as ctx:
        singles = ctx.enter_context(tc.tile_pool(name="singles", bufs=1))
        ldr = ctx.enter_context(tc.tile_pool(name="ldr", bufs=6))
        ph0 = ctx.enter_context(tc.tile_pool(name="ph0", bufs=1))
        phr = ctx.enter_context(tc.tile_pool(name="phr", bufs=2))
        rcp = ctx.enter_context(tc.tile_pool(name="rcp", bufs=2))
        ob = ctx.enter_context(tc.tile_pool(name="ob", bufs=3))
        # PSUM: 2 banks kv accum + 2x2-bank numA (double-buffered) +
        # 2-bank numB = 8 banks.
        pkv = ctx.enter_context(tc.tile_pool(name="pkv", bufs=2, space="PSUM"))
        pna = ctx.enter_context(tc.tile_pool(name="pna", bufs=2, space="PSUM"))
        pnb = ctx.enter_context(tc.tile_pool(name="pnb", bufs=1, space="PSUM"))

        sig_bias = singles.tile([P, 1], f32)
        nc.vector.memset(sig_bias, -4.102)

        # Block-diagonal kv operand per head pair: rows 0:64 cols 0:65 hold
        # kv_ext of the even head, rows 64:128 cols 65:130 the odd head.
        kv_bd = singles.tile([P, G, VW], f16)
        nc.vector.memset(kv_bd, 0.0)

        # Single Q staging tile so the load is 2 coarse scalar-queue
        # dispatches (pairs 0-1, pairs 2-3) costing ~1.8us of scalar
        # sequencer time instead of 4x ~0.9us.
        q_raw = singles.tile([P, G, L], f8)
        phiq = [singles.tile([P, L], f16, tag=f"pq{g}", name=f"pq{g}")
                for g in range(G)]
        # Pair 0's four KV batches share one tile: one 4096-elem phiK
        # activation covers the whole pair (PE is idle until it anyway).
        kvt0 = singles.tile([P, NB0, TB0, SUB, KVW], f8)

        kv_tiles = {}
        kv_ps = {}

        def emit_q_load(half):
            # Two coarse dispatches (pairs 0-1, 2-3) keep the scalar
            # sequencer cost low; they queue BEHIND kvr(1,0) on the
            # scalar HWDGE queue so the ramp-critical KV batch wins.
            nc.scalar.dma_start(
                out=q_raw[:, 2 * half:2 * half + 2, :],
                in_=Q[2 * half * P:(2 * half + 2) * P, :].rearrange(
                    "(k p) l -> p k l", k=2))

        def emit_kv_dispatch0(b):
            # b0-b2 on sync (fast-starting HWDGE); only b3 on gpsimd,
            # whose SWDGE queue delivers ~4us later than HWDGE.
            eng = nc.gpsimd if b == 3 else nc.sync
            eng.dma_start(
                out=kvt0[:, b, :, :, :],
                in_=KV0[b * P:(b + 1) * P, :].rearrange(
                    "p (t s e) -> p t s e", t=TB0, s=SUB))

        def emit_kv_dispatch_r(g, ib):
            r0 = ((g - 1) * NBS + ib) * P
            kvt = ldr.tile([P, TBS, SUB, KVW], f8, tag="kvtr", name="kvtr")
            # (1,0) rides the otherwise-idle scalar queue ahead of the Q
            # stream so pair-1 compute starts right after pair 0.
            if (g, ib) == (1, 0):
                eng = nc.scalar
            else:
                eng = nc.sync if ib == 0 else nc.gpsimd
            eng.dma_start(
                out=kvt,
                in_=KVR[r0:r0 + P, :].rearrange(
                    "p (t s e) -> p t s e", t=TBS, s=SUB))
            kv_tiles[(g, ib)] = kvt

        def emit_kv_compute0(b):
            # Per-batch activation so compute chases each landing batch.
            phiK = ph0.tile([P, TB0, SUB, GC], f16, tag="phiK0",
                            name="phiK0", bufs=2)
            nc.scalar.activation(
                out=phiK, in_=kvt0[:, b, :, :, 0:GC], func=SIG,
                bias=sig_bias, scale=0.6053)
            for t in range(TB0):
                for s in range(SUB):
                    nc.tensor.matmul(
                        out=kv_ps[0][:, 0:VW],
                        lhsT=phiK[:, t, s, :],
                        rhs=kvt0[:, b, t, s, GC:KVW],
                        start=(b == 0 and t == 0 and s == 0),
                        stop=(b == NB0 - 1 and t == TB0 - 1
                              and s == SUB - 1))

        def emit_kv_compute(g, ib):
            kvt = kv_tiles.pop((g, ib))
            phiK = phr.tile([P, TBS, SUB, GC], f16, tag="phiKr", name="phiK")
            nc.scalar.activation(
                out=phiK, in_=kvt[:, :, :, 0:GC], func=SIG, bias=sig_bias,
                scale=0.6053)
            for t in range(TBS):
                for s in range(SUB):
                    nc.tensor.matmul(
                        out=kv_ps[g][:, 0:VW],
                        lhsT=phiK[:, t, s, :],
                        rhs=kvt[:, t, s, GC:KVW],
                        start=(ib == 0 and t == 0 and s == 0),
                        stop=(ib == NBS - 1 and t == TBS - 1
                              and s == SUB - 1))

        def emit_phiq(g, qb=None):
            # qb=None: whole pair in one activation (pairs 0-2); pair 3
            # stays split per piece so its Q matmuls chase the activation.
            sl = slice(0, L) if qb is None else slice(qb * QB, (qb + 1) * QB)
            nc.scalar.activation(
                out=phiq[g][:, sl], in_=q_raw[:, g, sl],
                func=SIG, bias=sig_bias, scale=0.6053)

        def emit_kv_finish(g):
            """Pack the pair's kv PSUM tile into the block-diag operand."""
            ps = kv_ps.pop(g)
            nc.vector.tensor_copy(out=kv_bd[0:D, g, 0:D], in_=ps[0:D, 0:D])
            nc.vector.tensor_copy(
                out=kv_bd[0:D, g, D:W], in_=ps[0:D, 2 * D:2 * D + 1])
            nc.vector.tensor_copy(
                out=kv_bd[D:P, g, W:W + D], in_=ps[D:P, D:2 * D])
            nc.vector.tensor_copy(
                out=kv_bd[D:P, g, W + D:VW], in_=ps[D:P, 2 * D:2 * D + 1])

        def emit_q_piece(g, qb, split_store=False):
            """16 q-block matmuls + batched recip/mult + store.

            num blocks live 3-per-bank in two double-buffered 2-bank
            tiles (6 blocks each) and one 2-bank tile (4 blocks, 2 per
            bank); one reciprocal and one 5-dim-AP multiply per group.
            Double-buffered numA means the next piece's matmuls never
            wait on this piece's multiplies (tail-critical)."""
            pi = g * NQB + qb
            q0 = qb * QB
            orow = pi * P
            out_t = ob.tile([P, QB], f16, tag="outt", name="out_t")
            engs = ([nc.sync, nc.gpsimd] if pi % 2 == 0
                    else [nc.gpsimd, nc.sync])

            def group(blk0, nk, num, r, c0):
                for i in range(nk * 3):
                    k, j = divmod(i, 3)
                    nc.tensor.matmul(
                        out=sub_ap(num, 512 * k + 130 * j, [[1, VW]]),
                        lhsT=phiq[g][:, q0 + (blk0 + i) * P:
                                     q0 + (blk0 + i + 1) * P],
                        rhs=kv_bd[:, g, :], start=True, stop=True)
                nc.vector.reciprocal(
                    out=r, in_=sub_ap(num, D, [[512, nk], [130, 3], [W, 2]]))
                nc.vector.tensor_tensor(
                    out=sub_ap(out_t, c0,
                               [[384, nk], [128, 3], [64, 2], [1, 64]]),
                    in0=sub_ap(num, 0, [[512, nk], [130, 3], [W, 2], [1, D]]),
                    in1=sub_ap(r, 0, [[6, nk], [2, 3], [1, 2], [0, D]]),
                    op=mybir.AluOpType.mult)
                if split_store:
                    engs[(c0 // 768) % 2].dma_start(
                        out=O[orow:orow + P, c0:c0 + nk * 384],
                        in_=out_t[:, c0:c0 + nk * 384])

            for h in range(2):
                numA = pna.tile([P, 1024], f32, tag="numA", name="numA")
                rA = rcp.tile([P, 2, 3, 2], f32, tag="rA", name="rA")
                group(6 * h, 2, numA, rA, 768 * h)

            numB = pnb.tile([P, 1024], f32, tag="numB", name="numB")
            for blk in range(12, 16):
                k, j = divmod(blk - 12, 2)
                nc.tensor.matmul(
                    out=sub_ap(numB, 512 * k + 130 * j, [[1, VW]]),
                    lhsT=phiq[g][:, q0 + blk * P:q0 + (blk + 1) * P],
                    rhs=kv_bd[:, g, :], start=True, stop=True)
            rB = rcp.tile([P, 2, 2, 2], f32, tag="rB", name="rB")
            nc.vector.reciprocal(
                out=rB, in_=sub_ap(numB, D, [[512, 2], [130, 2], [W, 2]]))
            nc.vector.tensor_tensor(
                out=sub_ap(out_t, 1536, [[256, 2], [128, 2], [64, 2], [1, 64]]),
                in0=sub_ap(numB, 0, [[512, 2], [130, 2], [W, 2], [1, D]]),
                in1=sub_ap(rB, 0, [[4, 2], [2, 2], [1, 2], [0, D]]),
                op=mybir.AluOpType.mult)
            if split_store:
                engs[0].dma_start(out=O[orow:orow + P, 1536:QB],
                                  in_=out_t[:, 1536:QB])
            else:
                engs[0].dma_start(out=O[orow:orow + P, :], in_=out_t)

        # Dummy sigmoid on the bias column: pulls the framework's
        # activation-table load to the head of the scalar stream where it
        # overlaps the idle ramp (the load is async; only the first
        # dependent activation waits on it).
        warm = singles.tile([P, 1], f32)
        nc.scalar.activation(out=warm, in_=sig_bias, func=SIG,
                             bias=sig_bias, scale=1.0)

        # ---- load dispatches.  Per-engine streams:
        #   scalar: kvr(1,0), QA (pairs 0-1), QB (pairs 2-3)
        #   sync:   kv0 b0/b2, kvr(2,0), kvr(3,0), even stores
        #   gpsimd: kv0 b1/b3, kvr(1,1), kvr(2,1), kvr(3,1), odd stores
        emit_kv_dispatch_r(1, 0)
        emit_q_load(0)
        emit_q_load(1)
        for b in range(NB0):
            emit_kv_dispatch0(b)
        emit_kv_dispatch_r(1, 1)
        for g in range(2, G):
            for ib in range(NBS):
                emit_kv_dispatch_r(g, ib)

        # ---- software-pipelined compute: pair g's kv streams while
        # pair g-1's Q phase computes.  phiQ(g-1) interleaves with pair
        # g's phiK so the scalar stream never starves the PE. ----
        kv_ps[0] = pkv.tile([P, 512], f32, tag="kv", name="kv")
        for b in range(NB0):
            emit_kv_compute0(b)
        emit_kv_finish(0)
        for g in range(1, G):
            kv_ps[g] = pkv.tile([P, 512], f32, tag="kv", name="kv")
            emit_kv_compute(g, 0)
            emit_phiq(g - 1, 0)
            emit_q_piece(g - 1, 0)
            emit_kv_compute(g, 1)
            emit_phiq(g - 1, 1)
            emit_q_piece(g - 1, 1)
            emit_kv_finish(g)
        emit_phiq(G - 1, 0)
        emit_q_piece(G - 1, 0, split_store=True)
        emit_phiq(G - 1, 1)
        emit_q_piece(G - 1, 1, split_store=True)

    nc.compile()
    return nc


def _get_nc():
    if "nc" not in _CACHE:
        _CACHE["nc"] = _build_nc()
    return _CACHE["nc"]


def _f8():
    import ml_dtypes
    return ml_dtypes.float8_e3m4


def _shard_q(arr):
    """Full [B, L, E] f32 -> per-core transposed [512, L] fp8 slices."""
    f8 = _f8()
    out = []
    for c in range(N_CORES):
        b, g = divmod(c, 2)
        out.append(np.ascontiguousarray(
            arr[b, :, g * EC:(g + 1) * EC].T.astype(f8)))
    return out


def _shard_kv(karr, varr):
    """Full K/V [B, L, E] f32 -> per-core (KV0, KVR) fp8 staging: merged
    [K_pair(128) | V_pair(128) | 1 | 1] rows pre-permuted into SBUF tile
    order [ib][p][t][s][e] so every batch DMA is one sequential sweep."""
    f8 = _f8()
    out = []
    for c in range(N_CORES):
        b, g = divmod(c, 2)
        ksl = karr[b, :, g * EC:(g + 1) * EC]
        vsl = varr[b, :, g * EC:(g + 1) * EC]
        st = np.ones((G, L, KVW), dtype=f8)
        for pg in range(G):
            st[pg, :, 0:GC] = ksl[:, pg * GC:(pg + 1) * GC].astype(f8)
            st[pg, :, GC:GC + P] = vsl[:, pg * P:(pg + 1) * P].astype(f8)
        # row (ib, t, p, s) -> staged position (ib, p, t, s)
        kv0 = st[0].reshape(NB0, TB0, P, SUB, KVW).transpose(
            0, 2, 1, 3, 4).reshape(NB0 * P, TB0 * SUB * KVW)
        kvr = st[1:].reshape(G - 1, NBS, TBS, P, SUB, KVW).transpose(
            0, 1, 3, 2, 4, 5).reshape((G - 1) * NBS * P, TBS * SUB * KVW)
        out.append((np.ascontiguousarray(kv0), np.ascontiguousarray(kvr)))
    return out


def _unshard_o(o):
    """Per-core [4*2*128, 2048] fp16 (piece-major) -> [L, EC] f32 slice."""
    blocks = np.asarray(o).reshape(G, NQB, P, QB // P, P)  # [pg, qb, p, qk, e]
    # q = qb*QB + qk*P + p
    perm = blocks.transpose(0, 1, 3, 2, 4).reshape(G, L, P)
    return np.concatenate(list(perm), axis=1).astype(np.float32)


def make_in_maps(inputs):
    Q = np.asarray(inputs["Q"], dtype=np.float32)
    K = np.asarray(inputs["K"], dtype=np.float32)
    V = np.asarray(inputs["V"], dtype=np.float32)
    qs, kvs = _shard_q(Q), _shard_kv(K, V)
    return [{"Q": qs[c], "KV0": kvs[c][0], "KV": kvs[c][1]}
            for c in range(N_CORES)]


def run_sharded(in_maps, trace=False, trace_cores=None):
    from concourse.bass_utils import run_bass_kernel_spmd

    nc = _get_nc()
    kwargs = {}
    if trace:
        kwargs = dict(trace=True, trace_cores=trace_cores or [0])
    return run_bass_kernel_spmd(nc, in_maps, core_ids=list(range(N_CORES)),
                                **kwargs)


def kernel(**inputs):
    in_maps = make_in_maps(inputs)
    res = run_sharded(in_maps)
    out = np.empty((B, L, E), dtype=np.float32)
    for c in range(N_CORES):
        b, g = divmod(c, 2)
        out[b, :, g * EC:(g + 1) * EC] = _unshard_o(res.results[c]["O"])
    return out
